# revision 4
# baseline (speedup 1.0000x reference)
"""Trainium2 Bass kernel v2: 2-layer LSTM decoder + vocab projection + log-softmax.

Differences vs v1 baseline:
  - One vocab-quarter projection tile EVERY tick (fills the PE-idle AllGather
    window; phase-2 tail shrinks to ~5 tiles + last stats wave).
  - W_out resident in SBUF as fp8e4 (x128 scale) -> 62.5KB/partition; osb
    (projection stationary) cast to fp8 (x16) from the h1 ring. Descale 1/2048
    folded into the exp bias-scale and the passB tensor_scalar.
  - osb accumulated in SBUF directly from the AllGather output (no outs_d).
  - Softmax normalizer AllReduces run in 4 waves (8 groups each) DURING the
    recurrence; pass-B (logp = logits/S - logZ) spread 1 quarter-chunk/tick.
  - Sigmoid computed as 0.5*tanh(x/2)+0.5 so scalar engine uses only
    {tanh, identity, exp, ln} -> single activation table, zero reloads.
  - ih1 batching D=4 (LAG=5), psg PSUM [128,1024] (2 banks) so the projection
    PSUM can double-buffer (2x2 banks).
"""

import numpy as np
import ml_dtypes
from contextlib import ExitStack

import concourse.bass as bass
import concourse.mybir as mybir
import concourse.tile as tile
from concourse import bacc
from concourse import bass_utils

F32 = mybir.dt.float32
BF16 = mybir.dt.bfloat16
FP8 = mybir.dt.float8e4
I32 = mybir.dt.int32
AF = mybir.ActivationFunctionType
ALU = mybir.AluOpType
bf16 = ml_dtypes.bfloat16
f8e4 = ml_dtypes.float8_e4m3

H = 1024
RH = 2048
V = 32000
B = 32
T = 128
NC = 8
GS = 1024          # gate rows per core per layer
HS = 256           # h dims per core
VS = V // NC       # vocab rows per core
VQ = VS // 4       # vocab quarter (1000)
D = 4              # layer-1 input-matmul batching (steps per weight pass)
LAG = D + 1        # layer-1 step lag behind layer 0
SOS_ID = 1

WSC = 128.0        # W_out fp8 scale
HSC = 16.0         # h1/osb fp8 scale
LSC = WSC * HSC    # logits scale in PSUM / logits_d

TS_FULL = T - 1    # 127 recurrence steps
NRING0 = 8         # h0 ring slots


def _p_major(w, kt, mt):
    """(kt*128, mt*128) -> (128, kt*mt*128) packed [p, k*mt*128 + m*128 + q]."""
    return np.ascontiguousarray(
        w.reshape(kt, 128, mt, 128).transpose(1, 0, 2, 3).reshape(128, kt * mt * 128)
    )


def _ktile_cols(a):
    """(kt*128, n) -> (128, kt*n) packed [p, k*n + j] = a[128k+p, j]."""
    kt = a.shape[0] // 128
    return np.ascontiguousarray(
        a.reshape(kt, 128, a.shape[1]).transpose(1, 0, 2).reshape(128, kt * a.shape[1])
    )


def prep_inputs(inp, ts=TS_FULL):
    """Host-side prep: slice/transpose/cast weights per core -> in_maps."""
    ntok_pad = ((ts * B + 127) // 128) * 128
    f32 = np.float32

    emb = np.asarray(inp["emb"], f32)
    tb = np.asarray(inp["target_batch"]).astype(np.int64)
    idx = tb[:, :ts].T.reshape(-1).astype(np.int32)       # (ts*B,) t-major
    idx = np.concatenate([idx, np.zeros(ntok_pad - idx.size, np.int32)])
    idx = np.ascontiguousarray(idx.reshape(ntok_pad // 128, 128).T)  # [p, group]

    ch = np.asarray(inp["context_h"], f32)
    cc = np.asarray(inp["context_c"], f32)
    h_init = np.concatenate([ch[0::2], ch[1::2]], axis=2)  # (2, B, RH)
    c_init = np.concatenate([cc[0::2], cc[1::2]], axis=2)

    def h_pack(hl):  # (B, RH) -> (128, 512) bf16 [p, 32k+b] = h[b, 128k+p]
        return np.ascontiguousarray(
            hl.T.reshape(16, 128, B).transpose(1, 0, 2).reshape(128, 16 * B)
        ).astype(bf16)

    Wih = [np.asarray(inp["W_ih0"], f32), np.asarray(inp["W_ih1"], f32)]
    Whh = [np.asarray(inp["W_hh0"], f32), np.asarray(inp["W_hh1"], f32)]
    bsum = [np.asarray(inp["b_ih0"], f32) + np.asarray(inp["b_hh0"], f32),
            np.asarray(inp["b_ih1"], f32) + np.asarray(inp["b_hh1"], f32)]
    W_out = np.asarray(inp["W_out"], f32)
    b_out = np.asarray(inp["b_out"], f32)

    in_maps = []
    # pre-activations of the g-gate are scaled x2 on the host (exact in bf16)
    # so the cell can apply ONE tanh(x*0.5) over all four gate chunks:
    # i,f,o get tanh(x/2) (the sigmoid half-trick), g gets tanh(x).
    gsc = np.ones((4 * HS, 1), np.float32)
    gsc[3 * HS:] = 2.0
    for c in range(NC):
        # gate rows for core c, in i,f,o,g chunk order (256 rows each)
        rows = np.concatenate([np.arange(RH * k + HS * c, RH * k + HS * (c + 1))
                               for k in (0, 1, 3, 2)])  # i,f,o,g
        wih0t = _p_major((Wih[0][rows] * gsc).T.astype(bf16), 8, 8)   # (128, 8192)
        whh0t = _p_major((Whh[0][rows] * gsc).T.astype(bf16), 16, 8)  # (128, 16384)
        wih1t = _p_major((Wih[1][rows] * gsc).T.astype(bf16), 16, 8)
        whh1t = _p_major((Whh[1][rows] * gsc).T.astype(bf16), 16, 8)
        b0 = np.ascontiguousarray(
            (bsum[0][rows] * gsc[:, 0]).reshape(8, 128).T)            # (128, 8)
        b1 = np.ascontiguousarray(
            (bsum[1][rows] * gsc[:, 0]).reshape(8, 128).T)
        wout8 = _ktile_cols(
            (W_out[VS * c:VS * (c + 1)].T * WSC).astype(f8e4))      # (128, 16*VS)
        boutc = (b_out[VS * c:VS * (c + 1)].reshape(1, VS) * LSC).astype(f8e4)

        def c_pack(cl):  # (B, RH) slice -> (128, 64) f32
            s = cl[:, HS * c:HS * (c + 1)].T  # (256, B)
            return np.ascontiguousarray(
                s.reshape(2, 128, B).transpose(1, 0, 2).reshape(128, 2 * B))

        in_maps.append({
            "idx": idx, "embt": emb,
            "wih0t": wih0t, "whh0t": whh0t, "wih1t": wih1t, "whh1t": whh1t,
            "b0": b0, "b1": b1, "wout8": wout8, "boutc": boutc,
            "h0init": h_pack(h_init[0]), "h1init": h_pack(h_init[1]),
            "c0init": c_pack(c_init[0]), "c1init": c_pack(c_init[1]),
        })
    return in_maps, ntok_pad


def build_nc(ts=TS_FULL):
    ntok_pad = ((ts * B + 127) // 128) * 128
    ntok = ts * B
    ngrp = ntok_pad // 128          # token groups of 128
    nticks = ts + LAG + 1

    nc = bacc.Bacc("TRN2", target_bir_lowering=False, debug=False,
                   enable_asserts=False, num_devices=NC)

    # ---- I/O ----
    idx_t = nc.dram_tensor("idx", [128, ntok_pad // 128], I32,
                           kind="ExternalInput").ap()
    emb_t = nc.dram_tensor("embt", [V, H], F32, kind="ExternalInput").ap()
    wih0_t = nc.dram_tensor("wih0t", [128, 8 * GS], BF16, kind="ExternalInput").ap()
    whh0_t = nc.dram_tensor("whh0t", [128, 16 * GS], BF16, kind="ExternalInput").ap()
    wih1_t = nc.dram_tensor("wih1t", [128, 16 * GS], BF16, kind="ExternalInput").ap()
    whh1_t = nc.dram_tensor("whh1t", [128, 16 * GS], BF16, kind="ExternalInput").ap()
    b0_t = nc.dram_tensor("b0", [128, 8], F32, kind="ExternalInput").ap()
    b1_t = nc.dram_tensor("b1", [128, 8], F32, kind="ExternalInput").ap()
    wout_t = nc.dram_tensor("wout8", [128, 16 * VS], FP8, kind="ExternalInput").ap()
    bout_t = nc.dram_tensor("boutc", [1, VS], FP8, kind="ExternalInput").ap()
    h0i_t = nc.dram_tensor("h0init", [128, 512], BF16, kind="ExternalInput").ap()
    h1i_t = nc.dram_tensor("h1init", [128, 512], BF16, kind="ExternalInput").ap()
    c0i_t = nc.dram_tensor("c0init", [128, 64], F32, kind="ExternalInput").ap()
    c1i_t = nc.dram_tensor("c1init", [128, 64], F32, kind="ExternalInput").ap()
    out_t = nc.dram_tensor("out", [ntok_pad, VS], F32, kind="ExternalOutput").ap()

    RG = [list(range(NC))]
    NW = (ngrp + 7) // 8            # stats waves (4)

    with ExitStack() as ctx:
        tc = ctx.enter_context(tile.TileContext(nc))
        dram = ctx.enter_context(tc.tile_pool(name="dram", bufs=1, space="DRAM"))
        agp = ctx.enter_context(tc.tile_pool(name="agp", bufs=6, space="DRAM"))
        arp = ctx.enter_context(tc.tile_pool(name="arp", bufs=2, space="DRAM"))
        keep = ctx.enter_context(tc.tile_pool(name="keep", bufs=1))

        # cross-phase SBUF
        m4 = keep.tile([128, 4 * ngrp], F32, tag="m4")
        s4 = keep.tile([128, 4 * ngrp], F32, tag="s4")
        m_all = keep.tile([128, ngrp], F32, tag="mall")
        s_all = keep.tile([128, ngrp], F32, tag="sall")
        logz = keep.tile([128, ngrp], F32, tag="logz")
        ones_s = keep.tile([1, 128], FP8, tag="ones")
        bout_s = keep.tile([1, VS], FP8, tag="bouts")

        # persistent DRAM
        xbf_d = dram.tile([ntok_pad, H], BF16, tag="xbf")
        g0_d = dram.tile([8, 128, ntok], BF16, tag="g0d")
        logits_d = dram.tile([ngrp, 128, VS], BF16, tag="logitsd")

        with tc.tile_pool(name="rp", bufs=1) as rp:
            # ---- recurrence-lifetime SBUF ----
            whh0_s = rp.tile([128, 16 * GS], BF16, tag="whh0s")
            b0_s = rp.tile([128, 8], F32, tag="b0s")
            b1_s = rp.tile([128, 8], F32, tag="b1s")
            h0ring = rp.tile([128, NRING0 * 512], BF16, tag="h0ring")
            h1ring = rp.tile([128, 2 * 512], BF16, tag="h1ring")    # 2 step slots
            g0ring = rp.tile([128, 2 * 1024], BF16, tag="g0ring")   # 2 blk x 4 steps
            g1ring = rp.tile([128, 1024], BF16, tag="g1ring")       # D steps [m,s,b]
            osb8 = rp.tile([128, 2 * 16 * 128], FP8, tag="osb8")    # 2 group bufs
            nc.gpsimd.memset(ones_s[:], 1.0)
            nc.gpsimd.memset(osb8[:], 0.0)
            nc.sync.dma_start(bout_s[:], bout_t[:])

            nc.sync.dma_start(whh0_s[:], whh0_t[:])
            nc.sync.dma_start(b0_s[:], b0_t[:])
            nc.sync.dma_start(b1_s[:], b1_t[:])
            # h inits live in the ring slots read at t=0 / j=0
            nc.sync.dma_start(h0ring[:, 512 * (NRING0 - 1):512 * NRING0], h0i_t[:])
            nc.sync.dma_start(h1ring[:, 512:1024], h1i_t[:])

            # ============ Phase 0: embeddings + G0 = X @ Wih0.T + b0 ============
            TH = 2048  # token half for XT chunking
            with tc.tile_pool(name="p0sb", bufs=2) as p0sb, \
                 tc.tile_pool(name="p0ev", bufs=2) as p0ev, \
                 tc.tile_pool(name="p0big", bufs=1) as p0big, \
                 tc.tile_pool(name="p0ps", bufs=2, space="PSUM") as p0ps:
                idxs = p0big.tile([128, ntok_pad // 128], I32, tag="idxs")
                nc.sync.dma_start(idxs[:], idx_t[:])
                for it in range(ntok_pad // 128):
                    xg = p0sb.tile([128, H], F32, tag="xg")
                    nc.gpsimd.indirect_dma_start(
                        out=xg[:], out_offset=None, in_=emb_t[:],
                        in_offset=bass.IndirectOffsetOnAxis(
                            ap=idxs[:, it:it + 1], axis=0))
                    xc = p0sb.tile([128, H], BF16, tag="xc")
                    nc.vector.tensor_copy(xc[:], xg[:])
                    nc.scalar.dma_start(xbf_d[128 * it:128 * (it + 1), :], xc[:])

                wih0_s = p0big.tile([128, 8 * GS], BF16, tag="wih0s")
                nc.sync.dma_start(wih0_s[:], wih0_t[:])
                xt_s = p0big.tile([128, 8 * TH], BF16, tag="xts")

                for half in range((ntok + TH - 1) // TH):
                    t0 = TH * half
                    tw = min(TH, ntok - t0)
                    twp = ((tw + 15) // 16) * 16  # transpose src rows mult of 16
                    for k in range(8):
                        nc.sync.dma_start_transpose(
                            xt_s[:, TH * k:TH * k + twp],
                            xbf_d[t0:t0 + twp, 128 * k:128 * (k + 1)])
                    nch = [(512 * i, min(512, tw - 512 * i))
                           for i in range((tw + 511) // 512)]
                    for m in range(8):
                        ps = p0ps.tile([128, 2048], F32, tag="p0ps")
                        for k in range(8):
                            lhs = wih0_s[:, k * GS + 128 * m: k * GS + 128 * (m + 1)]
                            for (o, w) in nch:
                                nc.tensor.matmul(
                                    ps[:, o:o + w], lhs,
                                    xt_s[:, TH * k + o: TH * k + o + w],
                                    start=(k == 0), stop=(k == 7))
                        ev = p0ev.tile([128, TH], BF16, tag="g0ev")
                        nc.scalar.activation(ev[:, :tw], ps[:, :tw], AF.Identity,
                                             bias=b0_s[:, m:m + 1])
                        nc.scalar.dma_start(g0_d[m, :, t0:t0 + tw], ev[:, :tw])

            # ============ Phase 1: recurrence + full projection interleave ======
            c_prev = [None, None]
            with tc.tile_pool(name="rp2", bufs=1) as rp2, \
                 tc.tile_pool(name="ps1", bufs=1, space="PSUM") as ps_pool, \
                 tc.tile_pool(name="psg1", bufs=1, space="PSUM") as psg1_pool, \
                 tc.tile_pool(name="psq", bufs=2, space="PSUM") as psq_pool, \
                 tc.tile_pool(name="pscr", bufs=2) as pscr_pool, \
                 tc.tile_pool(name="pbx", bufs=2) as pbx_pool, \
                 tc.tile_pool(name="cell", bufs=2) as cell_pool:

                # layer-1 weights: first needed at tick LAG, loaded after phase 0
                wih1_s = rp2.tile([128, 16 * GS], BF16, tag="wih1s")
                whh1_s = rp2.tile([128, 16 * GS], BF16, tag="whh1s")
                nc.scalar.dma_start(wih1_s[:], wih1_t[:])
                nc.scalar.dma_start(whh1_s[:], whh1_t[:])
                # full W_out resident in fp8
                wout_s = rp2.tile([128, 16 * VS], FP8, tag="wout8s")
                nc.scalar.dma_start(wout_s[:], wout_t[:])
                nchq = [(0, 512), (512, VQ - 512)]

                osb4 = osb8[:].rearrange("p (g k q) -> p g k q", g=2, k=16)

                def proj_tile_mm(p):
                    """Projection tile p matmuls: group p//4, vocab qtr p%4."""
                    g, q = p // 4, p % 4
                    v0 = VQ * q
                    psq = psq_pool.tile([128, 1024], F32, tag="psq")
                    for k in range(16):
                        lhs = osb4[:, g % 2, k, :]
                        for (o, w) in nchq:
                            nc.tensor.matmul(
                                psq[:, o:o + w], lhs,
                                wout_s[:, VS * k + v0 + o: VS * k + v0 + o + w],
                                start=(k == 0), stop=False)
                    for (o, w) in nchq:
                        nc.tensor.matmul(psq[:, o:o + w], ones_s[:, :],
                                         bout_s[:, v0 + o:v0 + o + w],
                                         start=False, stop=True)
                    return psq

                def proj_tile_drain(p, psq):
                    """Softmax stats + logits store for tile p (emitted a tick
                    later so the cell's vector ops aren't queued behind it)."""
                    g, q = p // 4, p % 4
                    gh = 4 * g + q
                    v0 = VQ * q
                    nc.vector.tensor_reduce(m4[:, gh:gh + 1], psq[:, :VQ],
                                            axis=mybir.AxisListType.X,
                                            op=ALU.max)
                    negm = pscr_pool.tile([128, 1], F32, tag="negm")
                    nc.vector.tensor_scalar_mul(negm[:], m4[:, gh:gh + 1],
                                                -1.0 / LSC)
                    # exp output is discarded (only accum_out matters) — write
                    # it into lsb's storage, then overwrite with the logits.
                    lsb = pscr_pool.tile([128, VQ], BF16, tag="lsb")
                    nc.scalar.activation(lsb[:], psq[:, :VQ], AF.Exp,
                                         bias=negm[:, :1], scale=1.0 / LSC,
                                         accum_out=s4[:, gh:gh + 1])
                    nc.vector.tensor_copy(lsb[:], psq[:, :VQ])
                    nc.scalar.dma_start(logits_d[g, :, v0:v0 + VQ], lsb[:])

                def combine_group(g):
                    """m_all/s_all[:, g] from the 4 quarters of group g."""
                    m4v = m4[:].rearrange("p (g q) -> p g q", q=4)
                    s4v = s4[:].rearrange("p (g q) -> p g q", q=4)
                    nc.vector.tensor_reduce(m_all[:, g:g + 1], m4v[:, g, :],
                                            axis=mybir.AxisListType.X,
                                            op=ALU.max)
                    dq = pscr_pool.tile([128, 4], F32, tag="dq")
                    nc.vector.tensor_scalar(
                        dq[:], m4v[:, g, :], m_all[:, g:g + 1], None,
                        op0=ALU.subtract)
                    nc.scalar.activation(dq[:], dq[:], AF.Exp, scale=1.0 / LSC)
                    nc.vector.tensor_mul(dq[:], dq[:], s4v[:, g, :])
                    nc.vector.tensor_reduce(s_all[:, g:g + 1], dq[:],
                                            axis=mybir.AxisListType.X,
                                            op=ALU.add)

                def wave_ar(w):
                    """AllReduce normalizer stats for groups [8w, 8w+8)."""
                    g0c = 8 * w
                    gw = min(8, ngrp - g0c)
                    mloc = arp.tile([128, 8], F32, tag="mloc")
                    mglob = arp.tile([128, 8], F32, tag="mglob",
                                     addr_space="Shared")
                    nc.sync.dma_start(mloc[:, :gw], m_all[:, g0c:g0c + gw])
                    nc.gpsimd.collective_compute(
                        "AllReduce", ALU.max, replica_groups=RG,
                        ins=[mloc[:].opt()], outs=[mglob[:].opt()])
                    mg_s = pbx_pool.tile([128, 8], F32, tag="mgs")
                    nc.sync.dma_start(mg_s[:, :gw], mglob[:, :gw])
                    dm = pbx_pool.tile([128, 8], F32, tag="dm")
                    nc.vector.tensor_sub(dm[:, :gw], m_all[:, g0c:g0c + gw],
                                         mg_s[:, :gw])
                    edm = pbx_pool.tile([128, 8], F32, tag="edm")
                    nc.scalar.activation(edm[:, :gw], dm[:, :gw], AF.Exp,
                                         scale=1.0 / LSC)
                    sp = pbx_pool.tile([128, 8], F32, tag="sp")
                    nc.vector.tensor_mul(sp[:, :gw], s_all[:, g0c:g0c + gw],
                                         edm[:, :gw])
                    sloc = arp.tile([128, 8], F32, tag="sloc")
                    sglob = arp.tile([128, 8], F32, tag="sglob",
                                     addr_space="Shared")
                    nc.sync.dma_start(sloc[:, :gw], sp[:, :gw])
                    nc.gpsimd.collective_compute(
                        "AllReduce", ALU.add, replica_groups=RG,
                        ins=[sloc[:].opt()], outs=[sglob[:].opt()])
                    sg_s = pbx_pool.tile([128, 8], F32, tag="sgs")
                    nc.sync.dma_start(sg_s[:, :gw], sglob[:, :gw])
                    lns = pbx_pool.tile([128, 8], F32, tag="lns")
                    nc.scalar.activation(lns[:, :gw], sg_s[:, :gw], AF.Ln)
                    nc.vector.tensor_scalar(
                        logz[:, g0c:g0c + gw], mg_s[:, :gw], 1.0 / LSC, None,
                        op0=ALU.mult)
                    nc.vector.tensor_add(logz[:, g0c:g0c + gw],
                                         logz[:, g0c:g0c + gw], lns[:, :gw])

                VH = VQ // 2

                def passb_chunk(g, hq):
                    """out rows of group g, vocab [VH*hq, VH*(hq+1)): logp."""
                    v0 = VH * hq
                    lin = pbx_pool.tile([128, VH], BF16, tag="lin")
                    nc.sync.dma_start(lin[:], logits_d[g, :, v0:v0 + VH])
                    lout = pbx_pool.tile([128, VH], F32, tag="lout")
                    nc.vector.tensor_scalar(
                        lout[:], lin[:], 1.0 / LSC, logz[:, g:g + 1],
                        op0=ALU.mult, op1=ALU.subtract)
                    nc.sync.dma_start(
                        out_t[128 * g:128 * (g + 1), v0:v0 + VH], lout[:])

                def g0_prefetch(blk):
                    """DMA G0 steps [D*blk, D*blk+D) -> g0ring half blk%2."""
                    t0 = D * blk
                    nsteps = min(D, ts - t0)
                    if nsteps <= 0:
                        return
                    dst = g0ring[:].rearrange("p (h m s b) -> p h m s b",
                                              h=2, m=8, b=B)
                    src = g0_d[:, :, B * t0: B * (t0 + nsteps)].rearrange(
                        "m p sb -> p m sb")
                    nc.scalar.dma_start(
                        dst[:, blk % 2, :, 0:nsteps, :].rearrange(
                            "p m s b -> p m (s b)"), src)

                def hh_matmul(w_s, rhs_ap, ps):
                    for m in range(8):
                        for k in range(16):
                            nc.tensor.matmul(
                                ps[:, B * m:B * (m + 1)],
                                w_s[:, k * GS + 128 * m: k * GS + 128 * (m + 1)],
                                rhs_ap[:, k, :],
                                start=(k == 0), stop=(k == 15))

                def cell(l, ps, gadd_ap):
                    """LSTM cell for layer l; returns hn tile [128, 64] bf16.

                    sigmoid(x) = 0.5*tanh(x/2)+0.5 keeps the scalar engine on
                    the tanh table (no ACT_TABLE_LOAD ping-pong with Exp).
                    """
                    g = cell_pool.tile([128, 256], F32, tag=f"g{l}")
                    nc.vector.tensor_add(
                        g[:].rearrange("p (m b) -> p m b", b=B),
                        ps[:, :256].rearrange("p (m b) -> p m b", b=B),
                        gadd_ap)
                    # one tanh covers all four chunks: g-gate pre-activations
                    # were host-scaled x2, so tanh(x*0.5) = tanh(x_orig).
                    th = cell_pool.tile([128, 256], BF16, tag=f"th{l}")
                    nc.scalar.activation(th[:], g[:], AF.Tanh, scale=0.5)
                    sfo = cell_pool.tile([128, 192], F32, tag=f"sfo{l}")
                    nc.vector.tensor_scalar(sfo[:], th[:, 0:192], 0.5, 0.5,
                                            op0=ALU.mult, op1=ALU.add)
                    t1 = cell_pool.tile([128, 64], F32, tag=f"t1{l}")
                    nc.vector.tensor_mul(t1[:], sfo[:, 0:64], th[:, 192:256])
                    t2 = cell_pool.tile([128, 64], F32, tag=f"t2{l}")
                    nc.vector.tensor_mul(t2[:], sfo[:, 64:128], c_prev[l][:])
                    cn = cell_pool.tile([128, 64], F32, tag=f"cn{l}")
                    nc.vector.tensor_add(cn[:], t1[:], t2[:])
                    c_prev[l] = cn
                    tcn = cell_pool.tile([128, 64], F32, tag=f"tc{l}")
                    nc.scalar.activation(tcn[:], cn[:], AF.Tanh)
                    hn = cell_pool.tile([128, 64], BF16, tag=f"hn{l}")
                    nc.vector.tensor_mul(hn[:], sfo[:, 128:192], tcn[:])
                    return hn

                c0s = cell_pool.tile([128, 64], F32, tag="cn0")
                nc.sync.dma_start(c0s[:], c0i_t[:])
                c_prev[0] = c0s
                c1s = cell_pool.tile([128, 64], F32, tag="cn1")
                nc.sync.dma_start(c1s[:], c1i_t[:])
                c_prev[1] = c1s

                g0_prefetch(0)
                g0_prefetch(1)

                h0r4 = h0ring[:].rearrange("p (s k b) -> p s k b", s=NRING0, b=B)
                h1r4 = h1ring[:].rearrange("p (s k b) -> p s k b", s=2, b=B)
                g0r5 = g0ring[:].rearrange("p (h m s b) -> p h m s b",
                                           h=2, m=8, b=B)
                g1r4 = g1ring[:].rearrange("p (m s b) -> p m s b", m=8, b=B)

                done_tiles = set()
                done_waves = set()
                done_pb = set()
                # wave w's last group (8w+7) combines at tick 32w+41 (tile
                # p=32w+31 mm at t=p+9, drain+combine the tick after); AR
                # right after, passB 2/tick.
                ar_ticks = {}
                pb_sched = {}
                for w in range(NW):
                    t_ar = 32 * w + 41
                    if t_ar >= nticks - 1:
                        continue
                    ar_ticks[t_ar] = w
                    tt = t_ar + 2
                    for i in range(64):
                        g, hq = 8 * w + i // 8, i % 8
                        while tt < nticks and len(pb_sched.get(tt, [])) >= 2:
                            tt += 1
                        if tt >= nticks:
                            break
                        pb_sched.setdefault(tt, []).append((g, hq))

                # one merged AllGather per tick: hn0(t) + hn1 of last tick
                agin_next = agp.tile([2, 2, 128, B], BF16, tag="agin")
                pending_drain = None
                for t in range(nticks):
                    j = t - LAG  # layer-1 step this tick
                    agin = agin_next
                    agin_next = agp.tile([2, 2, 128, B], BF16, tag="agin")

                    # ---- layer 0, step t ----
                    if t < ts:
                        ps0 = ps_pool.tile([128, 256], F32, tag="ps0")
                        rhs = h0r4[:, (t - 1) % NRING0, :, :]
                        hh_matmul(whh0_s, rhs, ps0)
                        if t % D == D - 1:
                            g0_prefetch(t // D + 2)
                        hn0 = cell(0, ps0, g0r5[:, (t // D) % 2, :, t % D, :])
                        nc.sync.dma_start(
                            agin[:, 0, :, :].rearrange("j p b -> p j b"),
                            hn0[:].rearrange("p (j b) -> p j b", b=B))

                    agout = agp.tile([NC, 2, 2, 128, B], BF16, tag="agout",
                                     addr_space="Shared")
                    nc.gpsimd.collective_compute(
                        "AllGather", ALU.bypass, replica_groups=RG,
                        ins=[agin[:].opt()], outs=[agout[:].opt()])
                    # ring copies split by r-halves across two idle DMA queues
                    # so descriptor processing parallelizes (64B chunks).
                    if t < ts:
                        nc.gpsimd.dma_start(
                            h0r4[:, t % NRING0, 0:8, :],
                            agout[0:4, :, 0].rearrange("r j p b -> p (r j) b"))
                        nc.sync.dma_start(
                            h0r4[:, t % NRING0, 8:16, :],
                            agout[4:8, :, 0].rearrange("r j p b -> p (r j) b"))
                    jj = t - 1 - LAG  # step whose h1 rides this AG
                    if 0 <= jj < ts:
                        nc.gpsimd.dma_start(
                            h1r4[:, jj % 2, 0:8, :],
                            agout[0:4, :, 1].rearrange("r j p b -> p (r j) b"))
                        nc.sync.dma_start(
                            h1r4[:, jj % 2, 8:16, :],
                            agout[4:8, :, 1].rearrange("r j p b -> p (r j) b"))
                        # accumulate h1 into the projection stationary (fp8)
                        gq, sq = jj // 4, jj % 4
                        nc.vector.tensor_scalar_mul(
                            osb4[:, gq % 2, :, B * sq:B * (sq + 1)],
                            h1r4[:, jj % 2, :, :], HSC)

                    # ---- layer 1, step j ----
                    if 0 <= j < ts:
                        ps1 = ps_pool.tile([128, 256], F32, tag="ps1t")
                        rhs = h1r4[:, (j - 1) % 2, :, :]
                        hh_matmul(whh1_s, rhs, ps1)
                        hn1 = cell(1, ps1, g1r4[:, :, j % D, :])
                        # hn1 rides the NEXT tick's AllGather
                        nc.sync.dma_start(
                            agin_next[:, 1, :, :].rearrange("j p b -> p j b"),
                            hn1[:].rearrange("p (j b) -> p j b", b=B))

                    # ---- ih1 batch for steps [t-4, t), emitted AFTER the L1
                    # section: its inputs (h0 slots t-4..t-1) completed last
                    # tick, so the 10us burst runs in THIS tick's AllGather
                    # window instead of sitting on the hn1 critical path at
                    # first-use time (one tick later).
                    jb = t - 4
                    if 0 <= jb < ts and jb % D == 0:
                        nb = min(D, ts - jb)
                        psg = psg1_pool.tile([128, 1024], F32, tag="psg1")
                        s0 = jb % NRING0
                        for m in range(8):
                            for k in range(16):
                                nc.tensor.matmul(
                                    psg[:, 128 * m: 128 * m + B * nb],
                                    wih1_s[:, k * GS + 128 * m:
                                           k * GS + 128 * (m + 1)],
                                    h0r4[:, s0:s0 + nb, k, :],
                                    start=(k == 0), stop=(k == 15))
                        for m in range(8):
                            nc.scalar.activation(
                                g1ring[:, 128 * m: 128 * m + B * nb],
                                psg[:, 128 * m: 128 * m + B * nb],
                                AF.Identity, bias=b1_s[:, m:m + 1])

                    # ---- drain last tick's projection, then this tick's mm --
                    if pending_drain is not None:
                        proj_tile_drain(*pending_drain)
                        if pending_drain[0] % 4 == 3:
                            combine_group(pending_drain[0] // 4)
                        pending_drain = None
                    p = j - 4
                    if 0 <= p < 4 * ngrp:
                        psq = proj_tile_mm(p)
                        pending_drain = (p, psq)
                        done_tiles.add(p)

                    # ---- stats wave AR + passB chunks ----
                    if t in ar_ticks:
                        wave_ar(ar_ticks[t])
                        done_waves.add(ar_ticks[t])
                    for (g, q) in pb_sched.get(t, []):
                        passb_chunk(g, q)
                        done_pb.add((g, q))

                # ============ tail: leftover tiles, last wave, passB ============
                if pending_drain is not None:
                    proj_tile_drain(*pending_drain)
                    if pending_drain[0] % 4 == 3:
                        combine_group(pending_drain[0] // 4)
                    pending_drain = None
                for p in range(4 * ngrp):
                    if p not in done_tiles:
                        g, q = p // 4, p % 4
                        if g == ngrp - 1 and q == 0:
                            # zero the pad-step slice of the last group's osb
                            pad0 = (ts % 4)
                            if pad0:
                                nc.gpsimd.memset(
                                    osb4[:, (ngrp - 1) % 2, :, B * pad0:], 0.0)
                        psq = proj_tile_mm(p)
                        proj_tile_drain(p, psq)
                        if p % 4 == 3:
                            combine_group(p // 4)
                for w in range(NW):
                    if w not in done_waves:
                        wave_ar(w)
                for g in range(ngrp):
                    for hq in range(8):
                        if (g, hq) not in done_pb:
                            passb_chunk(g, hq)

    nc.compile()
    return nc


_NC_CACHE = {}


def _get_nc(ts):
    if ts not in _NC_CACHE:
        _NC_CACHE[ts] = build_nc(ts)
    return _NC_CACHE[ts]


def run_device(inputs, ts=TS_FULL, **run_kwargs):
    in_maps, ntok_pad = prep_inputs(inputs, ts)
    nc = _get_nc(ts)
    res = bass_utils.run_bass_kernel_spmd(nc, in_maps,
                                          core_ids=list(range(NC)), **run_kwargs)
    ntok = ts * B
    logp = np.empty((ntok, V), np.float32)
    for c in range(NC):
        logp[:, VS * c:VS * (c + 1)] = res.results[c]["out"][:ntok]
    out = np.zeros((B, T, V), np.float32)
    out[:, 0, SOS_ID] = 1.0
    out[:, 1:1 + ts, :] = logp.reshape(ts, B, V).transpose(1, 0, 2)
    return out, res


def kernel(**inputs) -> np.ndarray:
    out, _ = run_device(inputs, TS_FULL)
    return out
